# revision 1
# baseline (speedup 1.0000x reference)
"""Trainium2 Bass kernel for LoopABMIL — flipped fp8 DoubleRow design.

reference:
    h = silu(x @ Wp + bp)            # [B, N, H]
    a = h @ Wa[:, 0] + ba            # [B, N]
    p = softmax(a masked to lengths) # [B, N]
    pooled = p @ h                   # [B, H]
    logits = pooled @ Wc + bc        # [B, C]

Device design (per core; softmax pooling is associative so each core
processes a slice of every bag's chunks and the host merges partials):

  - Projection runs "flipped": Wp is the stationary operand in fp8
    DoubleRow [128, 2, 128] k-pair slices; x streams as the moving
    operand in fp8 [128, 2, Sg] slabs.  Output is h-major z [h, patch]
    in PSUM, 2 h-halves per <=512-patch group.
  - silu via tanh (same ACT table set as exp -> zero table switches):
    h' = z * (1 + tanh(z/2)) = 2*silu(z).  ACT does tanh, DVE does the
    fused (t + 1) * z (scalar_tensor_tensor), writing hT fp8.  The 2x
    is undone by halving Wa on device and the pooled sums on the host.
  - bp folds into x on the host (x += u with Wp^T u = bp exactly); the
    ragged mask folds into crafted columns v with a(v) <= -60 so
    exp underflows to 0.  No bias or mask work on device.
  - Attention logits: one DoubleRow matmul per group with lhsT = Wa/2
    replicated across all 128 output columns -> a broadcast across
    partitions for free.  ACT exp (split at bag boundaries, accum_out
    = segment denominators) -> w_rep bf16.
  - Pooling: per (bag, h-half) scalar_tensor_tensor multiply-accumulate
    on DVE over the bag's contiguous patch range, emitted one group
    behind the main pipeline so exp/pooling never head-of-line-block
    the next group's tanh/silu in the strict-FIFO engine queues.
  - x is DMA'd small-chunks-first (fast start), then 1 MiB chunks (the
    SP engine pays ~0.6 us per DMA instruction issued).
"""

import sys

if "/opt/trn_rl_repo" not in sys.path:
    sys.path.insert(0, "/opt/trn_rl_repo")

from contextlib import ExitStack

import ml_dtypes
import numpy as np

import concourse.bacc as bacc
import concourse.tile as tile
from concourse import mybir
from concourse.bass_utils import run_bass_kernel_spmd

B, N, D, H, C = 8, 8192, 1024, 256, 2
P = 128
NCORES = 8
KP = 4            # DoubleRow k-pairs (contraction 4 * 256 = 1024 = D)
GS = 512          # patches per compute group (one PSUM bank of f32)
FP8 = mybir.dt.float8e4
E4 = ml_dtypes.float8_e4m3
BF = mybir.dt.bfloat16
F32 = mybir.dt.float32
DR = mybir.MatmulPerfMode.DoubleRow

_cache: dict = {}


def _groups(Np: int):
    """Compute groups (<=512): front-load two small ones for fast start."""
    sizes = []
    for want in (128, 384):
        if sum(sizes) + want <= Np:
            sizes.append(want)
    while Np - sum(sizes) > 0:
        sizes.append(min(GS, Np - sum(sizes)))
    offs = np.concatenate([[0], np.cumsum(sizes)[:-1]]).astype(int)
    return [(int(o), int(s)) for o, s in zip(offs, sizes)]


def _dma_chunks(Np: int):
    """x DMA chunks: small first (start fast), then ~1MiB (1024 patches)."""
    sizes = []
    for want in (128, 384, 512):
        if sum(sizes) + want <= Np:
            sizes.append(want)
    while Np - sum(sizes) > 0:
        sizes.append(min(1024, Np - sum(sizes)))
    offs = np.concatenate([[0], np.cumsum(sizes)[:-1]]).astype(int)
    return [(int(o), int(s)) for o, s in zip(offs, sizes)]


def _segments(Np: int, n_per_bag: tuple):
    """Split groups at bag boundaries: list of (off, len, group_idx, bag)."""
    bnd = np.cumsum(np.asarray(n_per_bag)) * P
    segs = []
    for gi, (off, sg) in enumerate(_groups(Np)):
        lo = off
        while lo < off + sg:
            b = int(np.searchsorted(bnd, lo, side="right"))
            hi = min(off + sg, int(bnd[b]) if b < len(bnd) else off + sg)
            segs.append((lo, hi - lo, gi, b))
            lo = hi
    return segs


def _build(G: int, n_per_bag: tuple) -> "bacc.Bacc":
    """n_per_bag is in on-device processing order."""
    Np = G * P
    segs = _segments(Np, n_per_bag)
    nseg = len(segs)
    chunks = _dma_chunks(Np)
    nc = bacc.Bacc("TRN2", target_bir_lowering=False)

    xpk = nc.dram_tensor("xpk", [P, G * D], FP8, kind="ExternalInput")
    # wblob slabs: 0-7 wp half0, 8-15 wp half1, 16-17 wa_rep pairs
    wblob = nc.dram_tensor("wblob", [P, 18 * P], FP8, kind="ExternalInput")
    out = nc.dram_tensor("out", [P, 16 + nseg], F32, kind="ExternalOutput")

    with tile.TileContext(nc) as tc, ExitStack() as ctx:
        const = ctx.enter_context(tc.tile_pool(name="const", bufs=1))
        xp = ctx.enter_context(tc.tile_pool(name="xp", bufs=1))
        tp = ctx.enter_context(tc.tile_pool(name="tp", bufs=3))
        store = ctx.enter_context(tc.tile_pool(name="store", bufs=1))
        outp = ctx.enter_context(tc.tile_pool(name="outp", bufs=1))
        zpool = ctx.enter_context(tc.tile_pool(name="zps", bufs=3, space="PSUM"))
        apool = ctx.enter_context(tc.tile_pool(name="aps", bufs=2, space="PSUM"))

        # HAM warm-up junk matmuls, gated only on a memset so they start
        # immediately and keep the PE busy until the first x chunk lands.
        warm_in = const.tile([P, P], BF, tag="warmin")
        nc.vector.memset(warm_in, 0.0)
        wps = apool.tile([P, GS], F32, tag="a")
        NWARM = 36
        for i in range(NWARM):
            nc.tensor.matmul(
                wps[:, 0:P], lhsT=warm_in, rhs=warm_in,
                start=(i == 0), stop=(i == NWARM - 1),
            )

        # x chunk DMAs first (first chunk is small -> PE starts early);
        # weights are tiny and slot in right after chunk 0's issue.
        xtiles = []
        for ci, (coff, csz) in enumerate(chunks):
            xg = xp.tile([P, 8, csz], FP8, tag=f"x{ci}")
            xtiles.append(xg)
            nc.sync.dma_start(
                out=xg, in_=xpk[:, coff * 8:(coff + csz) * 8]
            )
            if ci == 0:
                cb = const.tile([P, 18, P], FP8, tag="cblob")
                nc.sync.dma_start(out=cb, in_=wblob[:])

        hT = store.tile([P, 2, Np], FP8, tag="hT")
        w_rep = store.tile([P, Np], BF, tag="wrep")
        junk = store.tile([P, 8 * P], BF, tag="junk")
        zero_b = store.tile([P, 1], F32, tag="zerob")
        nc.vector.memset(zero_b, 0.0)
        out_sb = outp.tile([P, 16 + nseg], F32, tag="outsb")

        bnd = np.cumsum(np.asarray(n_per_bag)) * P

        def _emit_bag_pooling(b):
            """Pooling for bag b (STT multiply-accumulate on DVE), emitted
            right after the bag's last exp so the DVE FIFO runs it
            mid-stream rather than as an end tail."""
            s0 = 0 if b == 0 else int(bnd[b - 1])
            s1 = int(bnd[b])
            for half in (0, 1):
                nc.vector.scalar_tensor_tensor(
                    out=junk[:, 0:s1 - s0],
                    in0=hT[:, half, s0:s1],
                    scalar=1.0,
                    in1=w_rep[:, s0:s1],
                    op0=mybir.AluOpType.mult,
                    op1=mybir.AluOpType.mult,
                    accum_out=out_sb[:, 2 * b + half:2 * b + half + 1],
                )

        def _chunk_of(off):
            for ci, (coff, csz) in enumerate(chunks):
                if coff <= off < coff + csz:
                    return ci, off - coff
            raise AssertionError

        groups = _groups(Np)
        pooled_emitted = 0

        def _emit_exp_and_pooling(gi, off, sg, aps):
            nonlocal pooled_emitted
            for si, (soff, slen, sgi, _bag) in enumerate(segs):
                if sgi != gi:
                    continue
                lo2 = soff - off
                nc.scalar.activation(
                    out=w_rep[:, soff:soff + slen],
                    in_=aps[:, lo2:lo2 + slen],
                    func=mybir.ActivationFunctionType.Exp,
                    bias=zero_b[:, 0:1],
                    accum_out=out_sb[:, 16 + si:17 + si],
                )
            for b in range(B):
                if off < bnd[b] <= off + sg:
                    _emit_bag_pooling(b)
                    pooled_emitted += 1

        prev = None
        for gi, (off, sg) in enumerate(groups):
            ci, lo = _chunk_of(off)
            xg = xtiles[ci]
            assert lo + sg <= chunks[ci][1]
            zps = zpool.tile([P, 2, GS], F32, tag="z")
            for i in (0, 1):
                for j in range(KP):
                    nc.tensor.matmul(
                        zps[:, i, 0:sg],
                        lhsT=cb[:, 8 * i + 2 * j:8 * i + 2 * j + 2, :],
                        rhs=xg[:, 2 * j:2 * j + 2, lo:lo + sg],
                        start=(j == 0),
                        stop=(j == KP - 1),
                        perf_mode=DR,
                    )
            ts = tp.tile([P, 2, GS], BF, tag="t")
            nc.scalar.activation(
                out=ts[:, :, 0:sg], in_=zps[:, :, 0:sg],
                func=mybir.ActivationFunctionType.Tanh, scale=0.5,
                bias=zero_b[:, 0:1],
            )
            nc.vector.scalar_tensor_tensor(
                out=hT[:, :, off:off + sg],
                in0=ts[:, :, 0:sg],
                scalar=1.0,
                in1=zps[:, :, 0:sg],
                op0=mybir.AluOpType.add,
                op1=mybir.AluOpType.mult,
            )
            aps = apool.tile([P, GS], F32, tag="a")
            nc.tensor.matmul(
                aps[:, 0:sg],
                lhsT=cb[:, 16:18, :],
                rhs=hT[:, :, off:off + sg],
                start=True,
                stop=True,
                perf_mode=DR,
            )
            # one-group software pipeline: exp/pooling of the PREVIOUS
            # group, so they never head-of-line-block tanh/stt of this one
            if prev is not None:
                _emit_exp_and_pooling(*prev)
            prev = (gi, off, sg, aps)
        _emit_exp_and_pooling(*prev)
        # Relay the results through one DVE copy and one ACT copy before
        # the output DMA: each engine's strict FIFO guarantees the copy
        # runs after every accumulator write that engine made, so the DMA
        # (which depends on the relay tile through normal operand
        # tracking) can never race an accumulator drain.
        relay = outp.tile([P, 16 + nseg], F32, tag="relay")
        nc.vector.tensor_copy(relay[:, 0:16], out_sb[:, 0:16])
        nc.scalar.copy(relay[:, 16:16 + nseg], out_sb[:, 16:16 + nseg])
        nc.sync.dma_start(out=out[:], in_=relay)

    nc.compile()
    return nc


def _plan(lengths: np.ndarray):
    lens = np.asarray(lengths, dtype=np.int64)
    T = np.maximum((lens + P - 1) // P, 1)
    n = (T + NCORES - 1) // NCORES
    G = int(n.sum())
    return T, n, G


def _fold_vectors(Wp64, bp64, Wa64):
    """u: exact bp fold into x.  v: crafted column with a(v) << 0 (mask)."""
    A = Wp64.T @ Wp64
    u = Wp64 @ np.linalg.solve(A, bp64)
    z0 = np.where(Wa64[:, 0] < 0, 16.0, -16.0)
    v = Wp64 @ np.linalg.solve(A, z0 - bp64)
    return u, v


def _check_dummy(v, Wp, Wa_dev):
    """Emulate device math for the crafted column; return its logit a."""
    v8 = np.asarray(v, dtype=np.float32).astype(E4).astype(np.float64)
    Wp8 = np.asarray(Wp, dtype=np.float32).astype(E4).astype(np.float64)
    wa8 = np.asarray(Wa_dev, dtype=np.float64)  # already device-quantized
    z = v8 @ Wp8
    hp = z * (1.0 + np.tanh(z / 2.0))
    return float(hp @ wa8)


def _pack(x, lengths, u, v, T, n, G, perm):
    """Per-core xpk [128, G*1024] fp8, bags in perm order, slab-blocked
    per DMA chunk."""
    lens = np.asarray(lengths, dtype=np.int64)
    Np = G * P
    xs = np.asarray(x, dtype=np.float32) + u.astype(np.float32)[None, None, :]
    v32 = v.astype(np.float32)
    bs = np.concatenate([np.full(n[b], b) for b in perm])
    js = np.concatenate([np.arange(n[b]) for b in perm])
    in_maps = []
    for c in range(NCORES):
        ts = c + NCORES * js
        ts_clip = np.minimum(ts, T[bs] - 1)
        xc = xs[bs[:, None], ts_clip[:, None] * P + np.arange(P)[None, :], :]
        valid = np.clip(lens[bs] - ts * P, 0, P)
        invalid = np.arange(P)[None, :] >= valid[:, None]      # [G, 128]
        xc[invalid] = v32
        x8 = xc.astype(E4).reshape(Np, D)                      # [Np, 1024]
        xpk = np.empty((P, G * D), dtype=E4)
        for coff, csz in _dma_chunks(Np):
            blk = x8[coff:coff + csz].reshape(csz, 8, P).transpose(2, 1, 0)
            xpk[:, coff * 8:(coff + csz) * 8] = blk.reshape(P, 8 * csz)
        in_maps.append({"xpk": xpk})
    return in_maps


def _pack_weights(Wp, Wa):
    wblob = np.zeros((P, 18, P), dtype=E4)
    Wp8 = np.asarray(Wp, dtype=np.float32)
    for i in (0, 1):
        for s8 in range(8):
            # slab sigma = 8*i + s8 holds Wp[s8*128 + p, i*128 + m]
            wblob[:, 8 * i + s8, :] = Wp8[
                s8 * P:(s8 + 1) * P, i * P:(i + 1) * P
            ].astype(E4)
    wa_dev = (np.asarray(Wa, dtype=np.float32)[:, 0] / 2.0).astype(E4)
    for s in (0, 1):
        wblob[:, 16 + s, :] = np.tile(
            wa_dev[s * P:(s + 1) * P, None], (1, P)
        )
    return wblob.reshape(P, 18 * P), wa_dev


def _run(inputs: dict, trace: bool = False):
    x = np.asarray(inputs["x"], dtype=np.float32)
    lengths = np.asarray(inputs["lengths"])
    Wp = np.asarray(inputs["Wp"], dtype=np.float32)
    bp = np.asarray(inputs["bp"], dtype=np.float32)
    Wa = np.asarray(inputs["Wa"], dtype=np.float32)
    Wc = np.asarray(inputs["Wc"], dtype=np.float32)
    bc = np.asarray(inputs["bc"], dtype=np.float32)

    T, n, G = _plan(lengths)
    perm = np.arange(B)                          # natural bag order
    n_perm = tuple(int(v) for v in n[perm])
    key = (G, n_perm)
    if key not in _cache:
        _cache[key] = _build(G, n_perm)
    nc = _cache[key]

    u, v = _fold_vectors(
        Wp.astype(np.float64), bp.astype(np.float64), Wa.astype(np.float64)
    )
    wblob, wa_dev = _pack_weights(Wp, Wa)
    a_dummy = _check_dummy(v, Wp, wa_dev.astype(np.float32))
    assert a_dummy < -50.0, f"crafted mask column too weak: a={a_dummy}"

    in_maps = _pack(x, lengths, u, v, T, n, G, perm)
    for m in in_maps:
        m["wblob"] = wblob

    res = run_bass_kernel_spmd(
        nc, in_maps, core_ids=list(range(NCORES)), trace=trace
    )

    segs = _segments(G * P, n_perm)
    num = np.zeros((B, H), np.float64)
    den = np.zeros(B, np.float64)
    for r in res.results:
        o = r["out"].astype(np.float64)          # [128, 16 + nseg]
        for pos in range(B):
            b = int(perm[pos])
            num[b, 0:P] += o[:, 2 * pos]
            num[b, P:H] += o[:, 2 * pos + 1]
        for si, (_soff, _slen, _gi, pos) in enumerate(segs):
            den[int(perm[pos])] += o[0, 16 + si]
    pooled = num / (2.0 * den[:, None])
    logits = pooled @ Wc.astype(np.float64) + bc.astype(np.float64)
    return logits.astype(np.float32), res.exec_time_ns


def kernel(**inputs) -> np.ndarray:
    logits, _ = _run(inputs, trace=False)
    return logits

